# revision 5
# baseline (speedup 1.0000x reference)
"""GCN layer kernel for nn_GcnNet_17695265259748 on 8 trn2 NeuronCores.

out = A_norm @ mean_L(x) @ W + s*b

Distribution (source-sharded message passing):
  - Host computes xm = mean_L(x) (f32, 190ms) and cheap edge preprocessing
    (degrees, sym-norm weights, partition edges by SOURCE core, sort by
    destination block of 128, pad to fixed shapes).
  - Each core holds its 6250-row xm shard (bf16) and the edges whose source
    lives in that shard. For every global destination block (392 blocks over
    a zero-padded 50176 dest space) it gathers source rows from the LOCAL
    shard with one dma_gather (int16 local indices), builds a one-hot
    [edge, dest] matrix D = (iota==dstloc)*norm on DVE, and accumulates
    aggT[d,c] += D^T... via PE matmuls into PSUM; partial [50176,128] bf16.
  - ReduceScatter(add) combines the 8 partials; each core gets its
    6272-row destination shard.
  - Final: per 128-dest tile, transpose-load agg (bf16 DMA transpose) and
    matmul with W [128,300] plus a K=1 rank-1 update s*b; out shard f32.
  - Host concatenates shard outputs (dropping the 176 padded rows).

Falls back to a pure-numpy implementation if the device path fails.
"""

import math
import os

import numpy as np

N, L, C, F = 50000, 20, 128, 300
NCORES = 8
NPC_SRC = N // NCORES            # 6250 source rows per core
NB = 392                         # global dest blocks of 128 (padded space)
NP = NB * 128                    # 50176 padded dest space
NPC_DST = NP // NCORES           # 6272 dest rows per core
T_BLK_MIN = 5                    # min edge tiles of 128 per (core, block)

_RUNNER_CACHE = {}


def _host_prep(xm, edge_index):
    """Edge preprocessing on host. Returns per-core input arrays."""
    import ml_dtypes

    bf16 = ml_dtypes.bfloat16

    row = edge_index[0].astype(np.int32)
    col = edge_index[1].astype(np.int32)
    keep = row != col
    deg = np.bincount(row[keep], minlength=N).astype(np.float32) + 1.0
    dis = 1.0 / np.sqrt(deg)

    r = row[keep]
    c = col[keep]
    w = dis[r] * dis[c]
    loops = np.arange(N, dtype=np.int32)
    ra = np.concatenate([r, loops])
    ca = np.concatenate([c, loops])
    wa = np.concatenate([w, 1.0 / deg]).astype(np.float32)

    s = np.bincount(ca, weights=wa, minlength=N).astype(np.float32)
    s_pad = np.zeros(NP, dtype=np.float32)
    s_pad[:N] = s

    core_id = ra // NPC_SRC                      # by SOURCE
    blk = ca >> 7                                # global dest block
    key = core_id * NB + blk
    order = np.argsort(key, kind="stable")
    ks = key[order]
    src_local = (ra[order] - core_id[order] * NPC_SRC).astype(np.int16)
    dstloc = (ca[order] & 127).astype(np.float32)
    ws = wa[order]

    counts = np.bincount(ks, minlength=NCORES * NB)
    t_blk = max(T_BLK_MIN, int(math.ceil(counts.max() / 128)))
    ni = 128 * t_blk

    starts = np.zeros(NCORES * NB, dtype=np.int64)
    np.cumsum(counts[:-1], out=starts[1:])
    within = np.arange(ks.size, dtype=np.int64) - starts[ks]

    # dma_gather index layout: slot v -> [blk, v%16, v//16] int16, and the
    # 16-partition pattern must be replicated across all 128 partitions
    # (one copy per Q7 core).
    idx_arr = np.zeros((NCORES, NB, 16, ni // 16), dtype=np.int16)
    idx_off = ks * (16 * (ni // 16)) + (within % 16) * (ni // 16) + within // 16
    idx_arr.reshape(-1)[idx_off] = src_local
    idx_arr = np.ascontiguousarray(np.tile(idx_arr, (1, 1, 8, 1)))

    # gather data layout: slot v -> partition v%128, tile v//128
    off = ks * ni + (within % 128) * t_blk + within // 128
    dst_arr = np.zeros((NCORES, NB, 128, t_blk), dtype=bf16)
    dst_arr.reshape(-1)[off] = dstloc.astype(bf16)
    nrm_arr = np.zeros((NCORES, NB, 128, t_blk), dtype=bf16)
    nrm_arr.reshape(-1)[off] = ws.astype(bf16)

    xm_b = xm.astype(bf16)
    return xm_b, idx_arr, dst_arr, nrm_arr, s_pad, t_blk


def _build_runner(t_blk):
    """Build + compile the 8-core SPMD bass kernel. Returns a callable."""
    import concourse.bass as bass
    import concourse.tile as tile
    from concourse import bacc, mybir
    from concourse import bass_utils

    ni = 128 * t_blk
    bf = mybir.dt.bfloat16
    f32 = mybir.dt.float32
    i16 = mybir.dt.int16

    nc = bacc.Bacc(
        "TRN2",
        target_bir_lowering=False,
        debug=False,
        enable_asserts=False,
        num_devices=NCORES,
    )

    xm_d = nc.dram_tensor("xm", [NPC_SRC, C], bf, kind="ExternalInput").ap()
    idx_d = nc.dram_tensor("idx", [NB, 128, ni // 16], i16, kind="ExternalInput").ap()
    dst_d = nc.dram_tensor("dst", [NB, 128, t_blk], bf, kind="ExternalInput").ap()
    nrm_d = nc.dram_tensor("nrm", [NB, 128, t_blk], bf, kind="ExternalInput").ap()
    s_d = nc.dram_tensor("sv", [1, NPC_DST], bf, kind="ExternalInput").ap()
    w_d = nc.dram_tensor("wm", [C, F], bf, kind="ExternalInput").ap()
    b_d = nc.dram_tensor("bv", [1, F], bf, kind="ExternalInput").ap()
    out_d = nc.dram_tensor("out", [NPC_DST, F], f32, kind="ExternalOutput").ap()

    with tile.TileContext(nc) as tc:
        with (
            tc.tile_pool(name="const", bufs=1) as const,
            tc.tile_pool(name="sb", bufs=4) as sb,
            tc.tile_pool(name="eqp", bufs=4) as eqp,
            tc.tile_pool(name="ps", bufs=4, space="PSUM") as ps,
            tc.tile_pool(name="dram", bufs=1, space="DRAM") as dram,
        ):
            w_sb = const.tile([C, F], bf)
            nc.sync.dma_start(w_sb[:], w_d[:])
            b_sb = const.tile([1, F], bf)
            nc.sync.dma_start(b_sb[:], b_d[:])
            s_sb = const.tile([1, NPC_DST], bf)
            nc.sync.dma_start(s_sb[:], s_d[:])

            iota_i = const.tile([128, 128], i16)
            nc.gpsimd.iota(iota_i[:], pattern=[[1, 128]], base=0, channel_multiplier=0)
            iota_b = const.tile([128, 128], bf)
            nc.vector.tensor_copy(iota_b[:], iota_i[:])

            partial = dram.tile([NP, C], bf)
            agg = dram.tile([NPC_DST, C], bf)

            for b in range(NB):
                idx_sb = sb.tile([128, ni // 16], i16, tag="idx")
                nc.sync.dma_start(idx_sb[:], idx_d[b])
                dst_sb = sb.tile([128, t_blk], bf, tag="dst")
                nc.sync.dma_start(dst_sb[:], dst_d[b])
                nrm_sb = sb.tile([128, t_blk], bf, tag="nrm")
                nc.sync.dma_start(nrm_sb[:], nrm_d[b])

                g = sb.tile([128, t_blk, C], bf, tag="g")
                nc.gpsimd.dma_gather(
                    out_ap=g[:],
                    in_ap=xm_d[:],
                    idxs_ap=idx_sb[:],
                    num_idxs=ni,
                    num_idxs_reg=ni,
                    elem_size=C,
                )

                acc = ps.tile([128, C], f32, tag="acc")
                for t in range(t_blk):
                    eq = eqp.tile([128, 128], bf, tag="eq")
                    nc.vector.tensor_tensor(
                        out=eq[:],
                        in0=iota_b[:],
                        in1=dst_sb[:, t : t + 1].to_broadcast([128, 128]),
                        op=mybir.AluOpType.is_equal,
                    )
                    nc.vector.tensor_tensor(
                        out=eq[:],
                        in0=eq[:],
                        in1=nrm_sb[:, t : t + 1].to_broadcast([128, 128]),
                        op=mybir.AluOpType.mult,
                    )
                    nc.tensor.matmul(
                        acc[:],
                        lhsT=eq[:],
                        rhs=g[:, t, :],
                        start=(t == 0),
                        stop=(t == t_blk - 1),
                    )
                part_sb = sb.tile([128, C], bf, tag="part")
                nc.any.tensor_copy(part_sb[:], acc[:])
                nc.sync.dma_start(partial[b * 128 : (b + 1) * 128, :], part_sb[:])

            nc.gpsimd.collective_compute(
                "ReduceScatter",
                mybir.AluOpType.add,
                ins=[partial.opt()],
                outs=[agg.opt()],
                replica_groups=[list(range(NCORES))],
            )

            for lb in range(NPC_DST // 128):
                agg_t = sb.tile([128, 128], bf, tag="aggT")
                nc.sync.dma_start_transpose(agg_t[:], agg[lb * 128 : (lb + 1) * 128, :])
                opsum = ps.tile([128, F], f32, tag="ops")
                nc.tensor.matmul(opsum[:], lhsT=agg_t[:], rhs=w_sb[:], start=True, stop=False)
                nc.tensor.matmul(
                    opsum[:],
                    lhsT=s_sb[0:1, lb * 128 : (lb + 1) * 128],
                    rhs=b_sb[:],
                    start=False,
                    stop=True,
                )
                o_sb = sb.tile([128, F], f32, tag="osb")
                nc.any.tensor_copy(o_sb[:], opsum[:])
                nc.sync.dma_start(out_d[lb * 128 : (lb + 1) * 128, :], o_sb[:])

    nc.compile()

    def run(in_maps, trace=False, **kw):
        return bass_utils.run_bass_kernel_spmd(
            nc, in_maps, core_ids=list(range(NCORES)), trace=trace, **kw
        )

    return run


def _device_path(xm, edge_index, W, b, trace=False):
    import ml_dtypes

    bf16 = ml_dtypes.bfloat16
    xm_b, idx_arr, dst_arr, nrm_arr, s_pad, t_blk = _host_prep(xm, edge_index)

    if t_blk not in _RUNNER_CACHE:
        _RUNNER_CACHE[t_blk] = _build_runner(t_blk)
    run = _RUNNER_CACHE[t_blk]

    w_b = W.astype(bf16)
    b_b = b.reshape(1, F).astype(bf16)
    in_maps = []
    for p in range(NCORES):
        in_maps.append(
            {
                "xm": np.ascontiguousarray(xm_b[p * NPC_SRC : (p + 1) * NPC_SRC]),
                "idx": idx_arr[p],
                "dst": dst_arr[p],
                "nrm": nrm_arr[p],
                "sv": np.ascontiguousarray(
                    s_pad[p * NPC_DST : (p + 1) * NPC_DST].reshape(1, NPC_DST)
                ).astype(bf16),
                "wm": w_b,
                "bv": b_b,
            }
        )
    res = run(in_maps, trace=trace)
    outs = []
    for p in range(NCORES):
        lo = p * NPC_DST
        hi = min(lo + NPC_DST, N)
        outs.append(res.results[p]["out"][: hi - lo])
    return np.concatenate(outs, axis=0).astype(np.float32), res


def _numpy_fallback(xm, edge_index, W, b):
    row = edge_index[0].astype(np.int64)
    col = edge_index[1].astype(np.int64)
    keep = row != col
    deg = np.bincount(row[keep], minlength=N).astype(np.float32) + 1.0
    dis = 1.0 / np.sqrt(deg)
    r = row[keep]
    c = col[keep]
    w = dis[r] * dis[c]
    loops = np.arange(N, dtype=np.int64)
    ra = np.concatenate([r, loops])
    ca = np.concatenate([c, loops])
    wa = np.concatenate([w, 1.0 / deg]).astype(np.float32)
    s = np.bincount(ca, weights=wa, minlength=N).astype(np.float32)

    order = np.argsort(ca, kind="stable")
    cs = ca[order]
    msg = wa[order, None] * xm[ra[order]]
    bounds = np.searchsorted(cs, np.arange(N))
    agg = np.add.reduceat(msg, bounds, axis=0)
    agg[bounds == len(cs)] = 0.0
    # reduceat quirk: empty segments copy the next row; every node has a self
    # loop here so all segments are non-empty.
    return agg @ W + s[:, None] * b[None, :]


def kernel(x, edge_index, W, b):
    x = np.asarray(x)
    edge_index = np.asarray(edge_index)
    W = np.asarray(W, dtype=np.float32)
    b = np.asarray(b, dtype=np.float32)

    xm = x.mean(axis=1, dtype=np.float32)

    if os.environ.get("GCN_FORCE_NUMPY"):
        return _numpy_fallback(xm, edge_index, W, b).astype(np.float32)
    try:
        out, _ = _device_path(xm, edge_index, W, b)
        return out.astype(np.float32)
    except Exception:
        import traceback

        traceback.print_exc()
        return _numpy_fallback(xm, edge_index, W, b).astype(np.float32)


# revision 10
# speedup vs baseline: 1.2087x; 1.2087x over previous
"""GCN layer kernel for nn_GcnNet_17695265259748 on 8 trn2 NeuronCores.

out = A_norm @ mean_L(x) @ W + s*b

Distribution (source-sharded message passing):
  - Host computes xm = mean_L(x) (f32) and cheap edge preprocessing:
    degrees, dis = deg^-1/2, partition edges by SOURCE core, sort by
    destination block of 128. dis[src] is folded into the shipped xm rows
    (xm'[v] = dis[v]*xm[v], bf16) so the per-edge weight on device reduces
    to the destination factor dis[dst], applied once per dest block.
  - Each core: for every global destination block (392 blocks over a
    zero-padded 50176 dest space, grouped 8 blocks per dma_gather) it
    gathers source rows from its LOCAL 6250-row xm shard (int16 indices),
    builds a pure one-hot [edge, dest] matrix on DVE (pad slots use
    dstloc=200 which never matches), accumulates agg[d,c] via PE matmuls
    into PSUM, then applies dis[dst] during the PSUM->SBUF copy.
    Partial results form a [50176,128] bf16 tensor.
  - Two ReduceScatters (192 + 200 blocks) combine the 8 partials (the
    first overlaps with compute of the second half).
  - Final: per 128-dest tile, transpose-load agg (bf16 DMA transpose),
    matmul with W [128,300] plus a K=1 rank-1 update s*b; out bf16,
    host upcasts to f32 and reassembles.

Falls back to a pure-numpy implementation if the device path fails.
"""

import math
import os

import numpy as np

N, L, C, F = 50000, 20, 128, 300
NCORES = 8
NPC_SRC = N // NCORES            # 6250 source rows per core
NB = 392                         # global dest blocks of 128 (padded space)
NP = NB * 128                    # 50176 padded dest space
GATHER_TILE_CAP = 8              # max 128-slot tiles per dma_gather (1024 idxs)
NB1 = 192                        # blocks in first ReduceScatter
NB2 = NB - NB1                   # blocks in second ReduceScatter
SH1 = NB1 * 128 // NCORES        # 3072 rows/core from RS1
SH2 = NB2 * 128 // NCORES        # 3200 rows/core from RS2

_RUNNER_CACHE = {}


def _host_prep(xm, edge_index):
    """Edge preprocessing on host. Returns per-core input arrays."""
    import ml_dtypes

    bf16 = ml_dtypes.bfloat16

    row = edge_index[0].astype(np.int32)
    col = edge_index[1].astype(np.int32)
    keep = row != col
    deg = np.bincount(row[keep], minlength=N).astype(np.float32) + 1.0
    dis = 1.0 / np.sqrt(deg)

    r = row[keep]
    c = col[keep]
    loops = np.arange(N, dtype=np.int32)
    ra = np.concatenate([r, loops])
    ca = np.concatenate([c, loops])

    # s[dest] = sum of dis[src]*dis[dst] over incident edges (incl. self loop)
    s = np.bincount(ca, weights=dis[ra] * dis[ca], minlength=N).astype(np.float32)
    s_pad = np.zeros(NP, dtype=np.float32)
    s_pad[:N] = s
    dis_pad = np.zeros(NP, dtype=np.float32)
    dis_pad[:N] = dis
    # disT[d%128, d//128]: per-partition dest scale, one column per block
    disT = np.ascontiguousarray(dis_pad.reshape(NB, 128).T.astype(np.float32))

    core_id = ra // NPC_SRC                      # by SOURCE
    blk = ca >> 7                                # global dest block
    key = core_id * NB + blk
    order = np.argsort(key, kind="stable")
    ks = key[order]
    src_local = (ra[order] - core_id[order] * NPC_SRC).astype(np.int16)
    dstloc = (ca[order] & 127).astype(np.float32)

    counts = np.bincount(ks, minlength=NCORES * NB).reshape(NCORES, NB)
    tiles = np.maximum(1, (counts + 127) // 128)          # [NCORES, NB]
    # identical structure across cores so one NEFF serves all: take max
    tiles = np.maximum.reduce(tiles, axis=0)              # [NB]
    ntile_total = int(tiles.sum())
    slots_per_blk = tiles * 128
    blk_slot_start = np.zeros(NB, dtype=np.int64)
    np.cumsum(slots_per_blk[:-1], out=blk_slot_start[1:])
    tot_slots = int(slots_per_blk.sum())

    starts = np.zeros(NCORES * NB, dtype=np.int64)
    np.cumsum(counts.reshape(-1)[:-1], out=starts[1:])
    within = np.arange(ks.size, dtype=np.int64) - starts[ks]
    slot = blk_slot_start[ks % NB] + within               # slot in [0, tot_slots)

    # dma_gather index layout: slot v -> [16-part pattern v%16, v//16],
    # replicated across all 128 partitions (one copy per Q7 core).
    idx_arr = np.zeros((NCORES, 16, tot_slots // 16), dtype=np.int16)
    idx_off = (ks // NB) * (16 * (tot_slots // 16)) + (slot % 16) * (
        tot_slots // 16
    ) + slot // 16
    idx_arr.reshape(-1)[idx_off] = src_local
    idx_arr = np.ascontiguousarray(np.tile(idx_arr, (1, 8, 1)))

    # one-hot dest layout: slot v -> partition v%128, tile column v//128
    # pad slots keep dstloc=200 (never matches iota 0..127)
    dst_arr = np.full((NCORES, 128, ntile_total), 200.0, dtype=bf16)
    d_off = (ks // NB) * (128 * ntile_total) + (slot % 128) * ntile_total + (
        blk_slot_start[ks % NB] // 128 + within // 128
    )
    dst_arr.reshape(-1)[d_off] = dstloc.astype(bf16)

    # xm' = dis[src] * xm, bf16
    xm_b = (xm * dis[:, None]).astype(bf16)
    return xm_b, idx_arr, dst_arr, disT, s_pad, tiles


def _build_runner(tiles):
    """Build + compile the 8-core SPMD bass kernel. Returns a callable."""
    import concourse.bass as bass
    import concourse.tile as tile
    from concourse import bacc, mybir
    from concourse import bass_utils

    tiles = [int(t) for t in tiles]
    ntile_total = sum(tiles)
    tot_slots = ntile_total * 128
    bf = mybir.dt.bfloat16
    f32 = mybir.dt.float32
    i16 = mybir.dt.int16

    nc = bacc.Bacc(
        "TRN2",
        target_bir_lowering=False,
        debug=False,
        enable_asserts=False,
        num_devices=NCORES,
    )

    xm_d = nc.dram_tensor("xm", [NPC_SRC, C], bf, kind="ExternalInput").ap()
    idx_d = nc.dram_tensor("idx", [128, tot_slots // 16], i16, kind="ExternalInput").ap()
    dst_d = nc.dram_tensor("dst", [128, ntile_total], bf, kind="ExternalInput").ap()
    dis_d = nc.dram_tensor("disv", [128, NB], f32, kind="ExternalInput").ap()
    s_d = nc.dram_tensor("sv", [1, NP // NCORES], bf, kind="ExternalInput").ap()
    w_d = nc.dram_tensor("wm", [C, F], bf, kind="ExternalInput").ap()
    b_d = nc.dram_tensor("bv", [1, F], bf, kind="ExternalInput").ap()
    out_d = nc.dram_tensor("out", [NP // NCORES, F], bf, kind="ExternalOutput").ap()

    # group consecutive blocks into gathers of at most GATHER_TILE_CAP tiles
    # (dma_gather crashes beyond ~1024 indices per instruction)
    assert max(tiles) <= GATHER_TILE_CAP, tiles
    groups = []
    b0 = 0
    while b0 < NB:
        b1 = b0 + 1
        tot = tiles[b0]
        while b1 < NB and tot + tiles[b1] <= GATHER_TILE_CAP:
            tot += tiles[b1]
            b1 += 1
        groups.append((b0, b1))
        b0 = b1

    blk_tile0 = [0] * NB
    acc_t = 0
    for b in range(NB):
        blk_tile0[b] = acc_t
        acc_t += tiles[b]

    with tile.TileContext(nc) as tc:
        with (
            tc.tile_pool(name="const", bufs=1) as const,
            tc.tile_pool(name="sb", bufs=3) as sb,
            tc.tile_pool(name="eqp", bufs=6) as eqp,
            tc.tile_pool(name="ps", bufs=4, space="PSUM") as ps,
            tc.tile_pool(name="pso", bufs=2, space="PSUM") as pso,
            tc.tile_pool(name="dram", bufs=1, space="DRAM") as dram,
        ):
            w_sb = const.tile([C, F], bf)
            nc.sync.dma_start(w_sb[:], w_d[:])
            b_sb = const.tile([1, F], bf)
            nc.sync.dma_start(b_sb[:], b_d[:])
            s_sb = const.tile([1, NP // NCORES], bf)
            nc.sync.dma_start(s_sb[:], s_d[:])
            dis_sb = const.tile([128, NB], f32)
            nc.sync.dma_start(dis_sb[:], dis_d[:])

            iota_i = const.tile([128, 128], i16)
            nc.gpsimd.iota(iota_i[:], pattern=[[1, 128]], base=0, channel_multiplier=0)
            iota_b = const.tile([128, 128], bf)
            nc.vector.tensor_copy(iota_b[:], iota_i[:])

            partial = dram.tile([NP, C], bf)
            agg1 = dram.tile([SH1, C], bf)
            agg2 = dram.tile([SH2, C], bf)

            for (g0, g1) in groups:
                t0 = blk_tile0[g0]
                gt = blk_tile0[g1 - 1] + tiles[g1 - 1] - t0   # tiles in group
                ni = gt * 128
                idx_sb = sb.tile([128, ni // 16], i16, tag="idx")
                nc.sync.dma_start(idx_sb[:], idx_d[:, t0 * 8 : t0 * 8 + ni // 16])
                g = sb.tile([128, gt, C], bf, tag="g")
                nc.gpsimd.dma_gather(
                    out_ap=g[:],
                    in_ap=xm_d[:],
                    idxs_ap=idx_sb[:],
                    num_idxs=ni,
                    num_idxs_reg=ni,
                    elem_size=C,
                )
                dst_sb = sb.tile([128, gt], bf, tag="dst")
                nc.sync.dma_start(dst_sb[:], dst_d[:, t0 : t0 + gt])

                for b in range(g0, g1):
                    bt0 = blk_tile0[b] - t0
                    acc = ps.tile([128, C], f32, tag="acc")
                    for t in range(tiles[b]):
                        eq = eqp.tile([128, 128], bf, tag="eq")
                        nc.vector.tensor_tensor(
                            out=eq[:],
                            in0=iota_b[:],
                            in1=dst_sb[:, bt0 + t : bt0 + t + 1].to_broadcast([128, 128]),
                            op=mybir.AluOpType.is_equal,
                        )
                        nc.tensor.matmul(
                            acc[:],
                            lhsT=eq[:],
                            rhs=g[:, bt0 + t, :],
                            start=(t == 0),
                            stop=(t == tiles[b] - 1),
                        )
                    part_sb = sb.tile([128, C], bf, tag="part")
                    nc.vector.tensor_scalar(
                        out=part_sb[:],
                        in0=acc[:],
                        scalar1=dis_sb[:, b : b + 1],
                        scalar2=None,
                        op0=mybir.AluOpType.mult,
                    )
                    nc.sync.dma_start(partial[b * 128 : (b + 1) * 128, :], part_sb[:])

            nc.gpsimd.collective_compute(
                "ReduceScatter",
                mybir.AluOpType.add,
                ins=[partial[: NB1 * 128, :].opt()],
                outs=[agg1.opt()],
                replica_groups=[list(range(NCORES))],
            )
            nc.gpsimd.collective_compute(
                "ReduceScatter",
                mybir.AluOpType.add,
                ins=[partial[NB1 * 128 :, :].opt()],
                outs=[agg2.opt()],
                replica_groups=[list(range(NCORES))],
            )

            def emit_out(agg_ap, nblk, row0):
                for lb in range(nblk):
                    agg_t = sb.tile([128, 128], bf, tag="aggT")
                    nc.sync.dma_start_transpose(
                        agg_t[:], agg_ap[lb * 128 : (lb + 1) * 128, :]
                    )
                    opsum = pso.tile([128, F], f32, tag="ops")
                    nc.tensor.matmul(
                        opsum[:], lhsT=agg_t[:], rhs=w_sb[:], start=True, stop=False
                    )
                    r0 = row0 + lb * 128
                    nc.tensor.matmul(
                        opsum[:],
                        lhsT=s_sb[0:1, r0 : r0 + 128],
                        rhs=b_sb[:],
                        start=False,
                        stop=True,
                    )
                    o_sb = sb.tile([128, F], bf, tag="osb")
                    nc.vector.tensor_copy(o_sb[:], opsum[:])
                    nc.sync.dma_start(out_d[r0 : r0 + 128, :], o_sb[:])

            emit_out(agg1[:], SH1 // 128, 0)
            emit_out(agg2[:], SH2 // 128, SH1)

    nc.compile()

    def run(in_maps, trace=False, **kw):
        return bass_utils.run_bass_kernel_spmd(
            nc, in_maps, core_ids=list(range(NCORES)), trace=trace, **kw
        )

    run.nc = nc
    return run


def _make_in_maps(xm_b, idx_arr, dst_arr, disT, s_pad):
    import ml_dtypes

    bf16 = ml_dtypes.bfloat16
    in_maps = []
    for p in range(NCORES):
        sv = np.concatenate(
            [
                s_pad[p * SH1 : (p + 1) * SH1],
                s_pad[NB1 * 128 + p * SH2 : NB1 * 128 + (p + 1) * SH2],
            ]
        )
        in_maps.append(
            {
                "xm": None,  # filled below
                "idx": idx_arr[p],
                "dst": dst_arr[p],
                "disv": disT,
                "sv": np.ascontiguousarray(sv.reshape(1, -1)).astype(bf16),
                "wm": None,
                "bv": None,
            }
        )
    return in_maps


def _device_path(xm, edge_index, W, b, trace=False):
    import ml_dtypes

    bf16 = ml_dtypes.bfloat16
    xm_b, idx_arr, dst_arr, disT, s_pad, tiles = _host_prep(xm, edge_index)

    key = tuple(int(t) for t in tiles)
    if key not in _RUNNER_CACHE:
        _RUNNER_CACHE[key] = _build_runner(tiles)
    run = _RUNNER_CACHE[key]

    w_b = W.astype(bf16)
    b_b = b.reshape(1, F).astype(bf16)
    in_maps = _make_in_maps(xm_b, idx_arr, dst_arr, disT, s_pad)
    for p in range(NCORES):
        in_maps[p]["xm"] = np.ascontiguousarray(xm_b[p * NPC_SRC : (p + 1) * NPC_SRC])
        in_maps[p]["wm"] = w_b
        in_maps[p]["bv"] = b_b
    res = run(in_maps, trace=trace)
    return _assemble(res.results), res


def _assemble(results):
    out = np.empty((N, F), dtype=np.float32)
    for p in range(NCORES):
        o = results[p]["out"].astype(np.float32)
        lo1 = p * SH1
        hi1 = min(lo1 + SH1, N)
        if hi1 > lo1:
            out[lo1:hi1] = o[: hi1 - lo1]
        lo2 = NB1 * 128 + p * SH2
        hi2 = min(lo2 + SH2, N)
        if hi2 > lo2:
            out[lo2:hi2] = o[SH1 : SH1 + hi2 - lo2]
    return out


def _numpy_fallback(xm, edge_index, W, b):
    row = edge_index[0].astype(np.int64)
    col = edge_index[1].astype(np.int64)
    keep = row != col
    deg = np.bincount(row[keep], minlength=N).astype(np.float32) + 1.0
    dis = 1.0 / np.sqrt(deg)
    r = row[keep]
    c = col[keep]
    w = dis[r] * dis[c]
    loops = np.arange(N, dtype=np.int64)
    ra = np.concatenate([r, loops])
    ca = np.concatenate([c, loops])
    wa = np.concatenate([w, 1.0 / deg]).astype(np.float32)
    s = np.bincount(ca, weights=wa, minlength=N).astype(np.float32)

    order = np.argsort(ca, kind="stable")
    cs = ca[order]
    msg = wa[order, None] * xm[ra[order]]
    bounds = np.searchsorted(cs, np.arange(N))
    agg = np.add.reduceat(msg, bounds, axis=0)
    agg[bounds == len(cs)] = 0.0
    # reduceat quirk: empty segments copy the next row; every node has a self
    # loop here so all segments are non-empty.
    return agg @ W + s[:, None] * b[None, :]


def kernel(x, edge_index, W, b):
    x = np.asarray(x)
    edge_index = np.asarray(edge_index)
    W = np.asarray(W, dtype=np.float32)
    b = np.asarray(b, dtype=np.float32)

    xm = x.mean(axis=1, dtype=np.float32)

    if os.environ.get("GCN_FORCE_NUMPY"):
        return _numpy_fallback(xm, edge_index, W, b).astype(np.float32)
    try:
        out, _ = _device_path(xm, edge_index, W, b)
        return out.astype(np.float32)
    except Exception:
        import traceback

        traceback.print_exc()
        return _numpy_fallback(xm, edge_index, W, b).astype(np.float32)
